# revision 1
# baseline (speedup 1.0000x reference)
"""Masked dot-product attention (ESIM masked_softmax) Trainium2 Bass kernel.

Math (per batch):
    s   = q @ k^T ; t = s * m  (== q @ (k*m)^T, exact since m is 0/1)
    p   = exp(t) * m / sum_k(exp(t) * m)   (max-subtraction cancels; |s|<~50
                                            so exp() stays in fp32 range)
    out = p @ v = (exp(t) @ [v*m | m]) -> numerator | denominator

Device mapping (per core, 2 batches, data-parallel over 8 cores):
  - masked key rows are compacted away on the host (kept rows first, zero-mask
    padding to LKC=1792), shrinking every O(Lq*Lk) stage by ~12%.
  - scores are computed TRANSPOSED (k on partitions, q free) so exp(s^T) is
    directly the lhsT of the PV matmul; no O(Lq*Lk) transposes.
  - k*m / q are PE-transposed once per batch ([128,128] fp32 tiles), with q
    duplicated into both partition halves and k-blocks packed in pairs so the
    K=64 score matmuls row-tile two-at-a-time (~218ns per pair of N=512
    bf16 matmuls).
  - S matmul: 3 bf16 passes over hi/lo split operands (qh*kh + qh*kl + ql*kh)
    = within ~2^-16 of a full fp32 matmul at bf16 speed with LDWEIGHTS
    hidden. ATT_S_MODE=f32r selects a single fp22 pass instead.
  - PV uses float32r (fp22) with stationary [v*m | m]: column 64 of the
    accumulated output is the softmax denominator for free.
  - out^T [65, Lq] is PE-transposed back in 128-column chunks and normalized
    with a per-partition reciprocal multiply.
"""

import os
import sys

import numpy as np

sys.path.insert(0, "/opt/trn_rl_repo")

import concourse.bacc as bacc
import concourse.bass as bass
import concourse.mybir as mybir
import concourse.tile as tile
from concourse import bass_utils
from concourse.masks import make_identity

B, LQ, LK, D = 16, 2048, 2048, 64
NCORES = 8
PB = B // NCORES  # batches per core
P = 128
NQB = LQ // P  # 16 q-blocks

S_MODE = os.environ.get("ATT_S_MODE", "bf16_3p")  # "bf16_3p" | "f32r"
PV_MODE = os.environ.get("ATT_PV_MODE", "f32r")  # "f32r" | "fp32"
COMPACT = os.environ.get("ATT_COMPACT", "1") == "1"
LKC = 1792  # compacted key length (14 blocks); used when counts allow

F32 = mybir.dt.float32
F32R = mybir.dt.float32r
BF16 = mybir.dt.bfloat16
EXP = mybir.ActivationFunctionType.Exp


class _BatchCtx:
    pass


def _attention_core(tc, q_d, k_d, v_d, m_d, o_d, nkb):
    """Emit the per-core program. All dram handles are per-core shards."""
    nc = tc.nc
    npair = nkb // 2
    pools = []

    def pool(name, bufs, space="SBUF"):
        p = tc.alloc_tile_pool(name=name, bufs=bufs, space=space)
        pools.append(p)
        return p

    singles = pool("singles", 1)
    stage = pool("stage", 2)
    main = pool("main", 2)
    wtp = pool("wt", 12)
    outp = pool("outp", 2)
    smalls = pool("smalls", 4)

    ps_s = pool("ps_s", 3, space="PSUM")  # 3 x [128,1024] = 6 banks
    ps_pv = pool("ps_pv", 2, space="PSUM")  # 2 x 1-bank slots (pv chunks + nat)

    ident = singles.tile([P, P], F32, tag="ident")
    make_identity(nc, ident)

    three = S_MODE == "bf16_3p"
    sdt = F32 if three else F32R

    def prep_io(b, use_act_ring=False):
        bc = _BatchCtx()
        bc.m_sb = stage.tile([P, nkb], F32, tag="m", name=f"m_sb{b}")
        nc.sync.dma_start(out=bc.m_sb, in_=m_d[b].rearrange("(t p) -> p t", p=P))
        ksrc = k_d[b].rearrange("(t p) d -> p t d", p=P)
        bc.knat = stage.tile([P, nkb, D], F32, tag="knat", name=f"knat{b}")
        h0 = 2 * ((npair + 1) // 2)  # covers the k-pairs of transpose group 0
        nc.gpsimd.dma_start(out=bc.knat[:, :h0, :], in_=ksrc[:, :h0, :])
        nc.gpsimd.dma_start(out=bc.knat[:, h0:, :], in_=ksrc[:, h0:, :])
        bc.qdup = stage.tile([P, NQB, 2, D], F32, tag="qdup", name=f"qdup{b}")
        qsrc = q_d[b].rearrange("(t p) d -> p t d", p=P)
        if use_act_ring:
            # batch-0 head fast path: tiny dedicated loads for the first
            # k-pair and first 4 q-blocks so the first S matmul issues early.
            bc.kf = stage.tile([P, 2, D], F32, tag="kf", name=f"kf{b}")
            nc.sync.dma_start(out=bc.kf, in_=ksrc[:, 0:2, :])
            nc.sync.dma_start(out=bc.qdup[:, 0:4, 0, :], in_=qsrc[:, 0:4, :])
            nc.scalar.dma_start(out=bc.qdup[:, 0:4, 1, :], in_=qsrc[:, 0:4, :])
        for g in range(2):
            gs = slice(4 if (use_act_ring and g == 0) else g * 8, (g + 1) * 8)
            nc.sync.dma_start(out=bc.qdup[:, gs, 0, :], in_=qsrc[:, gs, :])
            # batch 0 head: second copy on the idle ACT HWDGE ring, parallel.
            # Later batches must not touch the ACT ring (it would stall exp).
            eng = nc.scalar if use_act_ring else nc.gpsimd
            eng.dma_start(out=bc.qdup[:, gs, 1, :], in_=qsrc[:, gs, :])
        bc.vnat = stage.tile([P, nkb, D], F32, tag="vnat", name=f"vnat{b}")
        nc.gpsimd.dma_start(out=bc.vnat, in_=v_d[b].rearrange("(t p) d -> p t d", p=P))
        return bc

    def prep_units(b, bc):
        """Closures emitting prep compute; callable in order, spreadable."""
        km = bc.knat  # host pre-multiplied: knat already is k*m
        bc.kmT = main.tile([P, npair, P], sdt, tag="kmT", name=f"kmT{b}")
        if three:
            bc.kmTh = main.tile([P, npair, P], BF16, tag="kmTh", name=f"kmTh{b}")
            bc.kmTl = main.tile([P, npair, P], BF16, tag="kmTl", name=f"kmTl{b}")
        bc.qT = main.tile([P, LQ], sdt, tag="qT", name=f"qT{b}")
        if three:
            bc.qTh = main.tile([P, LQ], BF16, tag="qTh", name=f"qTh{b}")
            bc.qTl = main.tile([P, LQ], BF16, tag="qTl", name=f"qTl{b}")
        bc.vme = stage.tile(
            [P, nkb, D + 1], F32R if PV_MODE == "f32r" else F32, tag="vme",
            name=f"vme{b}",
        )
        bc.out_sb = outp.tile([P, NQB, D], F32, tag="osb", name=f"osb{b}")

        h0 = 2 * ((npair + 1) // 2)
        fast = hasattr(bc, "kf")

        def u_fast():
            # first k-pair + first 4 q-blocks: unblocks S(j=0, c=0) early
            tr = ps_s.tile([P, 5 * P], F32, tag="s", name=f"trf{b}")
            nc.tensor.transpose(tr[:, 0:P], bc.kf, ident)
            for i in range(4):
                nc.tensor.transpose(
                    tr[:, (i + 1) * P : (i + 2) * P], bc.qdup[:, i], ident
                )
            dst = bc.kmT[:, 0:1, :].rearrange("p a b -> p (a b)")
            nc.scalar.copy(dst, tr[:, 0:P])
            nc.scalar.copy(bc.qT[:, 0:512], tr[:, P:])
            if three:
                dh = bc.kmTh[:, 0:1, :].rearrange("p a b -> p (a b)")
                nc.vector.tensor_copy(dh, dst)
                nc.vector.tensor_sub(
                    bc.kmTl[:, 0:1, :].rearrange("p a b -> p (a b)"), dst, dh
                )
                nc.vector.tensor_copy(bc.qTh[:, 0:512], bc.qT[:, 0:512])
                nc.vector.tensor_sub(
                    bc.qTl[:, 0:512], bc.qT[:, 0:512], bc.qTh[:, 0:512]
                )

        def u_kmT(grp, act_copy=False):
            jlo = grp * (npair + 1) // 2
            jhi = npair if grp else (npair + 1) // 2
            if fast and grp == 0:
                jlo = 1
            def go():
                nj = jhi - jlo
                tr = ps_s.tile([P, nj * P], F32, tag="s", name=f"trk{b}_{grp}")
                for j in range(jlo, jhi):
                    nc.tensor.transpose(
                        tr[:, (j - jlo) * P : (j - jlo + 1) * P],
                        km[:, 2 * j : 2 * j + 2, :], ident,
                    )
                dst = bc.kmT[:, jlo:jhi, :].rearrange("p a b -> p (a b)")
                (nc.scalar.copy if act_copy else nc.vector.tensor_copy)(dst, tr)
                if three:
                    dh = bc.kmTh[:, jlo:jhi, :].rearrange("p a b -> p (a b)")
                    nc.vector.tensor_copy(dh, dst)
                    nc.vector.tensor_sub(
                        bc.kmTl[:, jlo:jhi, :].rearrange("p a b -> p (a b)"),
                        dst, dh,
                    )
            return go

        def u_qT(g, ilo, ihi, act_copy=False):
            def go():
                tr = ps_s.tile(
                    [P, (ihi - ilo) * P], F32, tag="s", name=f"trq{b}_{g}_{ilo}"
                )
                for i in range(ilo, ihi):
                    t = g * 8 + i
                    nc.tensor.transpose(
                        tr[:, (i - ilo) * P : (i - ilo + 1) * P], bc.qdup[:, t], ident
                    )
                half = slice((g * 8 + ilo) * P, (g * 8 + ihi) * P)
                (nc.scalar.copy if act_copy else nc.vector.tensor_copy)(
                    bc.qT[:, half], tr
                )
                if three:
                    nc.vector.tensor_copy(bc.qTh[:, half], bc.qT[:, half])
                    nc.vector.tensor_sub(
                        bc.qTl[:, half], bc.qT[:, half], bc.qTh[:, half]
                    )
            return go

        def u_vme():
            nc.vector.tensor_copy(bc.vme[:, :, 0:D], bc.vnat)
            nc.vector.tensor_copy(bc.vme[:, :, D], bc.m_sb[:, :])

        units = [
            u_kmT(0, act_copy=fast), u_qT(0, 4 if fast else 0, 8, act_copy=fast),
            u_kmT(1), u_vme, u_qT(1, 0, 4), u_qT(1, 4, 8),
        ]
        if fast:
            units.insert(0, u_fast)
        else:
            units.insert(1, u_qT(0, 0, 4))
            units[2] = u_qT(0, 4, 8)
        return units

    def main_half(b, bc, h, side_work=(), finals_out=None):
        side = list(side_work)
        pvc = [
            ps_pv.tile([65, 512], F32, tag="pv", name=f"pv{b}_{h}_{c}")
            for c in range(2)
        ]
        if three:
            passes = [
                (bc.kmTh, bc.qTh, True, False),
                (bc.kmTl, bc.qTh, False, False),
                (bc.kmTh, bc.qTl, False, True),
            ]
        else:
            passes = [(bc.kmT, bc.qT, True, True)]
        wdt = F32R if PV_MODE == "f32r" else F32

        def emit_pv(j, wA, wB):
            # c innermost: consecutive matmuls alternate PSUM banks, so the
            # accumulate never waits on its own bank's drain.
            for kb, w in ((2 * j, wA), (2 * j + 1, wB)):
                for c in range(2):
                    cs = slice(c * 512, (c + 1) * 512)
                    nc.tensor.matmul(
                        pvc[c], bc.vme[:, kb, :], w[:, cs],
                        start=(kb == 0), stop=(kb == nkb - 1),
                    )

        pend = []
        for j in range(npair):
            sA = ps_s.tile([P, 1024], F32, tag="s", name=f"sA{b}_{h}_{j}")
            sB = ps_s.tile([P, 1024], F32, tag="s", name=f"sB{b}_{h}_{j}")
            # c innermost: consecutive same-side matmuls alternate banks (no
            # accumulate drain-wait) and A/B stay adjacent so they row-pair.
            for kt, qt, st, sp in passes:
                for c in range(2):
                    qs = slice(h * 1024 + c * 512, h * 1024 + (c + 1) * 512)
                    cs = slice(c * 512, (c + 1) * 512)
                    nc.tensor.matmul(
                        sA[:, cs], kt[0:64, j, :], qt[0:64, qs],
                        start=st, stop=sp, tile_position=(0, 0),
                    )
                    nc.tensor.matmul(
                        sB[:, cs], kt[64:128, j, :], qt[64:128, qs],
                        start=st, stop=sp, tile_position=(64, 0),
                    )
            wA = wtp.tile([P, 1024], wdt, tag="wt", name=f"wA{b}_{h}_{j}")
            wB = wtp.tile([P, 1024], wdt, tag="wt", name=f"wB{b}_{h}_{j}")
            nc.scalar.activation(out=wA, in_=sA, func=EXP)
            nc.scalar.activation(out=wB, in_=sB, func=EXP)
            # PV lags two j-groups: its exps finished long ago, so the
            # in-order PE never stalls on ScalarE here.
            pend.append((j, wA, wB))
            if len(pend) > 2:
                emit_pv(*pend.pop(0))
            if side:
                side.pop(0)()
        while pend:
            emit_pv(*pend.pop(0))
        while side:
            side.pop(0)()

        # drain: copy the accumulators out (freeing the pv slots for the next
        # half) and hand the transpose-back/normalize work to the caller so it
        # can interleave into the next half's stream instead of starving ACT.
        outT = outp.tile([D + 1, 1024], F32, tag="outT", name=f"outT{b}_{h}")
        for c in range(2):
            nc.vector.tensor_copy(outT[:, c * 512 : (c + 1) * 512], pvc[c])

        def fin(q0):
            def go():
                for qb in range(q0, q0 + 4):
                    nat = ps_s.tile([P, D + 1], F32, tag="s", name=f"nat{b}_{h}_{qb}")
                    nc.tensor.transpose(
                        nat, outT[:, qb * P : (qb + 1) * P],
                        ident[0 : D + 1, 0 : D + 1],
                    )
                    rc = smalls.tile([P, 1], F32, tag="rc", name=f"rc{b}_{h}_{qb}")
                    nc.vector.reciprocal(rc, nat[:, D : D + 1])
                    nc.vector.tensor_scalar_mul(
                        bc.out_sb[:, h * 8 + qb, :], nat[:, 0:D], rc
                    )
            return go

        if finals_out is None:
            fin(0)()
            fin(4)()
        else:
            finals_out.extend([fin(0), fin(4)])

    def store(b, bc):
        nc.sync.dma_start(
            out=o_d[b].rearrange("(t p) d -> p t d", p=P), in_=bc.out_sb
        )

    # Interleave batch 1's prep into batch 0's stream: no PE bubble at the
    # batch boundary, and prep transposes spread out so HAM stays warm.
    # Batch 1's input DMAs are issued after batch 0's so they don't delay
    # the head-critical loads on the shared rings. Only the units needed by
    # the first few S matmuls run before the main loop; the rest spread as
    # per-iteration side work so the PE reaches the first S matmul early.
    bcs = [prep_io(0, use_act_ring=True)]
    # u0: [fast, kmT0, qT0b, kmT1, vme, qT1a, qT1b]
    u0 = prep_units(0, bcs[0])
    for u in u0[:3]:
        u()
    if PB > 1:
        bcs.append(prep_io(1))
        # u1: [kmT0, qT0a, qT0b, kmT1, vme, qT1a, qT1b]
        u1 = prep_units(1, bcs[1])
    else:
        u1 = []
    side00 = [u0[4], u0[3], u0[5], u0[6]] + u1[:3]  # vme first (PV needs it)
    f = []
    main_half(0, bcs[0], 0, side_work=side00, finals_out=f)
    f2 = []
    main_half(0, bcs[0], 1, side_work=f + u1[3:], finals_out=f2)
    if PB > 1:
        f3 = []
        main_half(1, bcs[1], 0, side_work=f2, finals_out=f3)
        store(0, bcs[0])
        main_half(1, bcs[1], 1, side_work=f3, finals_out=None)
        store(1, bcs[1])
    else:
        for u in f2:
            u()
        store(0, bcs[0])

    for p in reversed(pools):
        p.release()


_NC_CACHE = {}


def _build_nc(nkb):
    if nkb in _NC_CACHE:
        return _NC_CACHE[nkb]
    lk = nkb * P
    nc = bacc.Bacc(None, target_bir_lowering=False, debug=False)
    q_d = nc.dram_tensor("q", [PB, LQ, D], F32, kind="ExternalInput")
    k_d = nc.dram_tensor("k", [PB, lk, D], F32, kind="ExternalInput")
    v_d = nc.dram_tensor("v", [PB, lk, D], F32, kind="ExternalInput")
    m_d = nc.dram_tensor("m", [PB, lk], F32, kind="ExternalInput")
    o_d = nc.dram_tensor("out", [PB, LQ, D], F32, kind="ExternalOutput")
    with tile.TileContext(nc) as tc:
        _attention_core(tc, q_d, k_d, v_d, m_d, o_d, nkb)
    nc.compile()
    _NC_CACHE[nkb] = nc
    return nc


def kernel(q, k, v, v_mask, _trace=False, _tmpdir=None):
    q = np.ascontiguousarray(q, dtype=np.float32)
    k = np.ascontiguousarray(k, dtype=np.float32)
    v = np.ascontiguousarray(v, dtype=np.float32)
    v_mask = np.ascontiguousarray(v_mask, dtype=np.float32)
    assert q.shape == (B, LQ, D), q.shape

    # fold the 0/1 mask into k and v on the host (exact; removes the device
    # DVE mask-multiply chain from the critical path)
    k = k * v_mask[:, :, None]
    v = v * v_mask[:, :, None]
    counts = (v_mask > 0.5).sum(axis=1)
    if COMPACT and counts.max() <= LKC:
        # kept key rows first (stable), zero-mask padding after; the packed
        # mask makes padded rows contribute exactly 0 on device.
        order = np.argsort(v_mask <= 0.5, axis=1, kind="stable")[:, :LKC]
        kk = np.take_along_axis(k, order[:, :, None], axis=1)
        vv = np.take_along_axis(v, order[:, :, None], axis=1)
        mm = np.take_along_axis(v_mask, order, axis=1)
        nkb = LKC // P
    else:
        kk, vv, mm = k, v, v_mask
        nkb = LK // P

    nc = _build_nc(nkb)
    in_maps = [
        {
            "q": np.ascontiguousarray(q[i * PB : (i + 1) * PB]),
            "k": np.ascontiguousarray(kk[i * PB : (i + 1) * PB]),
            "v": np.ascontiguousarray(vv[i * PB : (i + 1) * PB]),
            "m": np.ascontiguousarray(mm[i * PB : (i + 1) * PB]),
        }
        for i in range(NCORES)
    ]
    res = bass_utils.run_bass_kernel_spmd(
        nc, in_maps, core_ids=list(range(NCORES)), trace=_trace, tmpdir=_tmpdir
    )
    out = np.concatenate([r["out"] for r in res.results], axis=0)
    if _trace:
        kernel.last_results = res
    return out



# revision 2
# speedup vs baseline: 1.2352x; 1.2352x over previous
"""Masked dot-product attention (ESIM masked_softmax) Trainium2 Bass kernel.

Math (per batch):
    s   = q @ k^T ; t = s * m  (== q @ (k*m)^T, exact since m is 0/1)
    p   = exp(t) * m / sum_k(exp(t) * m)   (max-subtraction cancels; |s|<~50
                                            so exp() stays in fp32 range)
    out = p @ v = (exp(t) @ [v*m | m]) -> numerator | denominator

Device mapping (per core, 2 batch slots, data-parallel over 8 cores):
  - masked key rows are compacted away on the host (kept rows first, zero-mask
    padding); batches are sorted by kept-count so slot 0 takes the 8 largest
    (14 key blocks) and slot 1 the 8 smallest (13 blocks for the seed-0 data).
  - scores are computed TRANSPOSED (k on partitions, q free) so exp(s^T) is
    directly the lhsT of the PV matmul; no O(Lq*Lk) transposes.
  - k*m / q are PE-transposed once per slot and copied out of PSUM as fp16;
    the K=64 score matmuls run a SINGLE fp16 pass, row-tiled two-at-a-time.
    (fp16 q/k quantization adds ~5e-3 abs score error -> ~0.5% softmax error.)
  - exp() is split across the Scalar engine (native EXP activation) and the
    Vector engine (2-instruction custom-DVE Schraudolph exp, see dve_exp
    section) so neither engine is the sole bottleneck.
  - PV uses bf16 stationary [v*m | m]: column 64 of the accumulated output is
    the softmax denominator for free.
  - out^T [65, Lq] is PE-transposed back in 128-column chunks and normalized
    with a per-partition reciprocal multiply.
"""

import os
import sys

import numpy as np

sys.path.insert(0, "/opt/trn_rl_repo")

import concourse.bacc as bacc
import concourse.bass as bass  # noqa: F401
import concourse.mybir as mybir
import concourse.tile as tile
from concourse import bass_utils
from concourse import dve_ops
from concourse.dve_ops import DveOp
from concourse.dve_spec import AluOp, Bin, C0, C1, C2, Spec, Src0, Src1, maxx
from concourse.dve_spec import _has_src1
from concourse.dve_spec import lower as dve_lower
from concourse.dve_uop import DveOpSpec
from concourse.masks import make_identity

B, LQ, LK, D = 16, 2048, 2048, 64
NCORES = 8
PB = B // NCORES  # batch slots per core
P = 128
NQB = LQ // P  # 16 q-blocks

COMPACT = os.environ.get("ATT_COMPACT", "1") == "1"
# js whose B-side exp runs on the Vector engine (per half)
DVE_JS = tuple(
    int(x) for x in os.environ.get("ATT_DVE_JS", "1,3,5").split(",") if x != ""
)

F32 = mybir.dt.float32
F16 = mybir.dt.float16
BF16 = mybir.dt.bfloat16
I32 = mybir.dt.int32
EXP = mybir.ActivationFunctionType.Exp

# --------------------------------------------------------------------------
# Custom DVE exp (2 instructions, ~1.7e-3 max rel err).
# inst1 ANT_EXP_CRUDE: i32out = int(max(x*2^23*log2e + 127*2^23, 2^23))
#   -> Schraudolph bit pattern (127+n)<<23 | f<<23 via int32 convert-on-write.
# inst2 ANT_EXP_FIX reads it bitcast as fp32: crude = 2^n*(1+f):
#   p2n = AND(crude, s0-broadcast 0x7F800000) = 2^n exactly
#   nt  = NOT(p2n) = -(2-eps)*2^(1-n);  gp = crude*nt = -4*(1+f)*(1-2^-24)
#   out = p2n * (Src1 + gp*(imm2 + gp*s1))  [deg-2 minimax of 2^(-gp/4-1)]
# --------------------------------------------------------------------------
LOG2E_SCALED = float(2.0**23 / np.log(2.0))
EXP_BIAS = float(127.0 * 2.0**23)
EXP_FLOOR = float(1.0 * 2.0**23)
EXPMASK_BITS = 0x7F800000
_R0, _R1, _R2 = 0.68127841, -0.01673623, 0.33718486
P_R0 = float(_R0)  # Src1 fill (const term)
P_G2 = float(_R2 / 16.0)  # s1
P_G1 = float(-_R1 / 4.0)  # imm2


def _self_pinned(name, spec, row):
    if name in dve_ops._SUB_OPCODE_FOR_NAME:
        for op in dve_ops.OPS:
            if op.name == name:
                return op
    dve_ops._SUB_OPCODE_FOR_NAME[name] = row
    sha = {}
    for ver in ("v3", "v4"):
        uops = dve_lower(spec, ver=ver)
        sha[ver] = DveOpSpec(
            name=name, opcode=row, uops=uops, rd1_en=_has_src1(spec)
        ).sha(ver)
    op = DveOp(name, spec, subdim=False, uops_sha=sha)
    dve_ops.OPS.append(op)
    dve_ops.CUSTOM_DVE_SPECS[name] = spec
    return op


def _ref_crude(in0, in1, s0, s1, imm2):
    return np.maximum(
        in0.astype(np.float32) * np.float32(s0) + np.float32(s1), np.float32(imm2)
    )


def _ref_fix(in0, in1, s0, s1, imm2):
    ci = np.ascontiguousarray(in0, dtype=np.float32).view(np.int32)
    mask_i = np.ascontiguousarray(np.asarray(s0, np.float32).reshape(-1, 1)).view(
        np.int32
    )
    p2n_bits = ci & mask_i
    p2n = p2n_bits.view(np.float32)
    nt = (~p2n_bits).view(np.float32)
    gp = in0.astype(np.float32) * nt
    r0 = in1.astype(np.float32).reshape(in0.shape)
    return p2n * (r0 + gp * (np.float32(imm2) + gp * np.float32(s1)))


EXP_CRUDE = _self_pinned(
    "ANT_EXP_CRUDE", Spec(body=maxx(Src0 * C0 + C1, C2), reference=_ref_crude), row=17
)
_p2n = Bin(AluOp.BITWISE_AND, Src0, C0)
_nt = Bin(AluOp.BITWISE_NOT, _p2n, _p2n)
_gp = Src0 * _nt
EXP_FIX = _self_pinned(
    "ANT_EXP_FIX",
    Spec(body=_p2n * (Src1 + _gp * (C2 + _gp * C1)), reference=_ref_fix),
    row=18,
)


class _BatchCtx:
    pass


def _attention_core(tc, q_d, k_ds, v_ds, m_ds, o_d, nkbs):
    """Emit the per-core program. k/v/m dram handles and nkb per batch slot."""
    nc = tc.nc
    pools = []

    def pool(name, bufs, space="SBUF"):
        p = tc.alloc_tile_pool(name=name, bufs=bufs, space=space)
        pools.append(p)
        return p

    singles = pool("singles", 1)
    stage = pool("stage", 2)
    main = pool("main", 2)
    wtp = pool("wt", 12)
    crp = pool("crp", 2)
    outp = pool("outp", 2)
    smalls = pool("smalls", 4)

    ps_s = pool("ps_s", 3, space="PSUM")  # 3 x [128,1024] = 6 banks
    ps_pv = pool("ps_pv", 2, space="PSUM")  # 2 x 1-bank slots (pv chunks + nat)

    ident = singles.tile([P, P], F32, tag="ident")
    make_identity(nc, ident)
    emask = singles.tile([P, 1], I32, tag="emask")
    nc.gpsimd.memset(emask, EXPMASK_BITS)
    emask_f = emask.bitcast(F32)
    r0t = singles.tile([P, 1024], F32, tag="r0t")
    nc.gpsimd.memset(r0t, P_R0)

    def emit_exp_dve(w_ap, s_ap, tag):
        crude = crp.tile([P, 1024], I32, tag="crude", name=f"cr{tag}")
        nc.vector._custom_dve(
            EXP_CRUDE, out=crude, in0=s_ap,
            s0=LOG2E_SCALED, s1=EXP_BIAS, imm2=EXP_FLOOR,
        )
        nc.vector._custom_dve(
            EXP_FIX, out=w_ap, in0=crude.bitcast(F32),
            in1=r0t, s0=emask_f, s1=P_G2, imm2=P_G1,
        )

    def prep_io(b, use_act_ring=False):
        nkb = nkbs[b]
        npair = (nkb + 1) // 2
        bc = _BatchCtx()
        bc.nkb, bc.npair = nkb, npair
        bc.m_sb = stage.tile([P, nkb], F32, tag="m", name=f"m_sb{b}")
        nc.sync.dma_start(out=bc.m_sb, in_=m_ds[b].rearrange("(t p) -> p t", p=P))
        ksrc = k_ds[b].rearrange("(t p) d -> p t d", p=P)
        bc.knat = stage.tile([P, nkb, D], F32, tag="knat", name=f"knat{b}")
        h0 = min(2 * ((npair + 1) // 2), nkb)
        nc.gpsimd.dma_start(out=bc.knat[:, :h0, :], in_=ksrc[:, :h0, :])
        if h0 < nkb:
            nc.gpsimd.dma_start(out=bc.knat[:, h0:, :], in_=ksrc[:, h0:, :])
        bc.qdup = stage.tile([P, NQB, 2, D], F32, tag="qdup", name=f"qdup{b}")
        qsrc = q_d[b].rearrange("(t p) d -> p t d", p=P)
        if use_act_ring:
            # slot-0 head fast path: tiny dedicated loads for the first
            # k-pair and first 4 q-blocks so the first S matmul issues early.
            bc.kf = stage.tile([P, 2, D], F32, tag="kf", name=f"kf{b}")
            nc.sync.dma_start(out=bc.kf, in_=ksrc[:, 0:2, :])
            nc.sync.dma_start(out=bc.qdup[:, 0:4, 0, :], in_=qsrc[:, 0:4, :])
            nc.scalar.dma_start(out=bc.qdup[:, 0:4, 1, :], in_=qsrc[:, 0:4, :])
        for g in range(2):
            gs = slice(4 if (use_act_ring and g == 0) else g * 8, (g + 1) * 8)
            nc.sync.dma_start(out=bc.qdup[:, gs, 0, :], in_=qsrc[:, gs, :])
            # slot-0 head: second copy on the idle ACT HWDGE ring, parallel.
            # Later slots must not touch the ACT ring (it would stall exp).
            eng = nc.scalar if use_act_ring else nc.gpsimd
            eng.dma_start(out=bc.qdup[:, gs, 1, :], in_=qsrc[:, gs, :])
        bc.vnat = stage.tile([P, nkb, D], F32, tag="vnat", name=f"vnat{b}")
        nc.gpsimd.dma_start(out=bc.vnat, in_=v_ds[b].rearrange("(t p) d -> p t d", p=P))
        return bc

    def prep_units(b, bc):
        """Closures emitting prep compute; callable in order, spreadable."""
        nkb, npair = bc.nkb, bc.npair
        km = bc.knat  # host pre-multiplied: knat already is k*m
        bc.kmT = main.tile([P, npair, P], F16, tag="kmT", name=f"kmT{b}")
        bc.qT = main.tile([P, LQ], F16, tag="qT", name=f"qT{b}")
        bc.vme = stage.tile([P, nkb, D + 1], BF16, tag="vme", name=f"vme{b}")
        bc.out_sb = outp.tile([P, NQB, D], F32, tag="osb", name=f"osb{b}")

        h0 = 2 * ((npair + 1) // 2)
        fast = hasattr(bc, "kf")

        def u_fast():
            # first k-pair + first 4 q-blocks: unblocks S(j=0, c=0) early
            tr = ps_s.tile([P, 5 * P], F32, tag="s", name=f"trf{b}")
            nc.tensor.transpose(tr[:, 0:P], bc.kf, ident)
            for i in range(4):
                nc.tensor.transpose(
                    tr[:, (i + 1) * P : (i + 2) * P], bc.qdup[:, i], ident
                )
            dst = bc.kmT[:, 0:1, :].rearrange("p a b -> p (a b)")
            nc.scalar.copy(dst, tr[:, 0:P])
            nc.scalar.copy(bc.qT[:, 0:512], tr[:, P:])

        def u_kmT(grp, act_copy=False):
            jlo = grp * (npair + 1) // 2
            jhi = npair if grp else (npair + 1) // 2
            if fast and grp == 0:
                jlo = 1

            def go():
                nj = jhi - jlo
                if nj <= 0:
                    return
                tr = ps_s.tile([P, nj * P], F32, tag="s", name=f"trk{b}_{grp}")
                for j in range(jlo, jhi):
                    blk = slice(2 * j, min(2 * j + 2, nkb))
                    rows = slice(0, 64 * (blk.stop - blk.start))
                    nc.tensor.transpose(
                        tr[rows, (j - jlo) * P : (j - jlo + 1) * P],
                        km[:, blk, :], ident,
                    )
                dst = bc.kmT[:, jlo:jhi, :].rearrange("p a b -> p (a b)")
                (nc.scalar.copy if act_copy else nc.vector.tensor_copy)(dst, tr)

            return go

        def u_qT(g, ilo, ihi, act_copy=False):
            def go():
                tr = ps_s.tile(
                    [P, (ihi - ilo) * P], F32, tag="s", name=f"trq{b}_{g}_{ilo}"
                )
                for i in range(ilo, ihi):
                    t = g * 8 + i
                    nc.tensor.transpose(
                        tr[:, (i - ilo) * P : (i - ilo + 1) * P], bc.qdup[:, t], ident
                    )
                half = slice((g * 8 + ilo) * P, (g * 8 + ihi) * P)
                (nc.scalar.copy if act_copy else nc.vector.tensor_copy)(
                    bc.qT[:, half], tr
                )

            return go

        def u_vme():
            nc.gpsimd.tensor_copy(bc.vme[:, :, 0:D], bc.vnat)
            nc.gpsimd.tensor_copy(bc.vme[:, :, D], bc.m_sb[:, :])

        units = [
            u_kmT(0, act_copy=fast), u_qT(0, 4 if fast else 0, 8, act_copy=fast),
            u_kmT(1), u_vme, u_qT(1, 0, 4), u_qT(1, 4, 8),
        ]
        if fast:
            units.insert(0, u_fast)
        else:
            units.insert(1, u_qT(0, 0, 4))
            units[2] = u_qT(0, 4, 8)
        return units

    def main_half(b, bc, h, side_work=(), finals_out=None):
        nkb, npair = bc.nkb, bc.npair
        odd = nkb % 2  # last pair has only an A block
        side = list(side_work)
        pvc = [
            ps_pv.tile([D + 1, 512], F32, tag="pv", name=f"pv{b}_{h}_{c}")
            for c in range(2)
        ]

        def emit_pv(j, wA, wB):
            # c innermost: consecutive matmuls alternate PSUM banks, so the
            # accumulate never waits on its own bank's drain.
            kbs = [(2 * j, wA)]
            if wB is not None:
                kbs.append((2 * j + 1, wB))
            for kb, w in kbs:
                for c in range(2):
                    cs = slice(c * 512, (c + 1) * 512)
                    nc.tensor.matmul(
                        pvc[c], bc.vme[:, kb, :], w[:, cs],
                        start=(kb == 0), stop=(kb == nkb - 1),
                    )

        pend = []
        for j in range(npair):
            jodd = odd and j == npair - 1
            sA = ps_s.tile([P, 1024], F32, tag="s", name=f"sA{b}_{h}_{j}")
            sB = None if jodd else ps_s.tile([P, 1024], F32, tag="s", name=f"sB{b}_{h}_{j}")
            # c innermost: consecutive same-side matmuls alternate banks (no
            # accumulate drain-wait) and A/B stay adjacent so they row-pair.
            for c in range(2):
                qs = slice(h * 1024 + c * 512, h * 1024 + (c + 1) * 512)
                cs = slice(c * 512, (c + 1) * 512)
                nc.tensor.matmul(
                    sA[:, cs], bc.kmT[0:64, j, :], bc.qT[0:64, qs],
                    start=True, stop=True, tile_position=(0, 0),
                )
                if not jodd:
                    nc.tensor.matmul(
                        sB[:, cs], bc.kmT[64:128, j, :], bc.qT[64:128, qs],
                        start=True, stop=True, tile_position=(64, 0),
                    )
            wA = wtp.tile([P, 1024], BF16, tag="wt", name=f"wA{b}_{h}_{j}")
            wB = None if jodd else wtp.tile([P, 1024], BF16, tag="wt", name=f"wB{b}_{h}_{j}")
            # B-side exp of designated js runs on the Vector engine; emitting
            # it first keeps the ACT stream dense. PV lags two j-groups so the
            # in-order PE never stalls on exp latency here.
            if (not jodd) and j in DVE_JS:
                emit_exp_dve(wB, sB, f"{b}_{h}_{j}")
                nc.scalar.activation(out=wA, in_=sA, func=EXP)
            else:
                nc.scalar.activation(out=wA, in_=sA, func=EXP)
                if not jodd:
                    nc.scalar.activation(out=wB, in_=sB, func=EXP)
            pend.append((j, wA, wB))
            if len(pend) > 2:
                emit_pv(*pend.pop(0))
            if side:
                side.pop(0)()
        while pend:
            emit_pv(*pend.pop(0))
        while side:
            side.pop(0)()

        # drain: copy the accumulators out (freeing the pv slots for the next
        # half) and hand the transpose-back/normalize work to the caller so it
        # can interleave into the next half's stream instead of starving ACT.
        outT = outp.tile([D + 1, 1024], F32, tag="outT", name=f"outT{b}_{h}")
        for c in range(2):
            nc.vector.tensor_copy(outT[:, c * 512 : (c + 1) * 512], pvc[c])

        def fin(q0):
            def go():
                for qb in range(q0, q0 + 4):
                    nat = ps_s.tile([P, D + 1], F32, tag="s", name=f"nat{b}_{h}_{qb}")
                    nc.tensor.transpose(
                        nat, outT[:, qb * P : (qb + 1) * P],
                        ident[0 : D + 1, 0 : D + 1],
                    )
                    rc = smalls.tile([P, 1], F32, tag="rc", name=f"rc{b}_{h}_{qb}")
                    nc.vector.reciprocal(rc, nat[:, D : D + 1])
                    nc.vector.tensor_scalar_mul(
                        bc.out_sb[:, h * 8 + qb, :], nat[:, 0:D], rc
                    )

            return go

        if finals_out is None:
            fin(0)()
            fin(4)()
        else:
            finals_out.extend([fin(0), fin(4)])

    def store(b, bc):
        nc.sync.dma_start(
            out=o_d[b].rearrange("(t p) d -> p t d", p=P), in_=bc.out_sb
        )

    # Interleave slot 1's prep into slot 0's stream: no PE bubble at the
    # boundary, and prep transposes spread out so HAM stays warm. Only the
    # units needed by the first few S matmuls run before the main loop; the
    # rest spread as per-iteration side work.
    bcs = [prep_io(0, use_act_ring=True)]
    # u0: [fast, kmT0, qT0b, kmT1, vme, qT1a, qT1b]
    u0 = prep_units(0, bcs[0])
    for u in u0[:3]:
        u()
    u0[4]()  # vme0 early (gpsimd; PV(j=0) needs it)
    if PB > 1:
        bcs.append(prep_io(1))
        # u1: [kmT0, qT0a, qT0b, kmT1, vme, qT1a, qT1b]
        u1 = prep_units(1, bcs[1])
    else:
        u1 = []
    side00 = [u0[3], u0[5], u0[6]] + u1[:3]
    f = []
    main_half(0, bcs[0], 0, side_work=side00, finals_out=f)
    f2 = []
    main_half(0, bcs[0], 1, side_work=f + u1[3:], finals_out=f2)
    if PB > 1:
        f3 = []
        main_half(1, bcs[1], 0, side_work=f2, finals_out=f3)
        store(0, bcs[0])
        main_half(1, bcs[1], 1, side_work=f3, finals_out=None)
        store(1, bcs[1])
    else:
        for u in f2:
            u()
        store(0, bcs[0])

    for p in reversed(pools):
        p.release()


_NC_CACHE = {}


def _build_nc(nkbs):
    nkbs = tuple(nkbs)
    if nkbs in _NC_CACHE:
        return _NC_CACHE[nkbs]
    nc = bacc.Bacc(None, target_bir_lowering=False, debug=False)
    q_d = nc.dram_tensor("q", [PB, LQ, D], F32, kind="ExternalInput")
    k_ds, v_ds, m_ds = [], [], []
    for s, nkb in enumerate(nkbs):
        lk = nkb * P
        k_ds.append(nc.dram_tensor(f"k{s}", [lk, D], F32, kind="ExternalInput"))
        v_ds.append(nc.dram_tensor(f"v{s}", [lk, D], F32, kind="ExternalInput"))
        m_ds.append(nc.dram_tensor(f"m{s}", [lk], F32, kind="ExternalInput"))
    o_d = nc.dram_tensor("out", [PB, LQ, D], F32, kind="ExternalOutput")
    with tile.TileContext(nc) as tc:
        _attention_core(tc, q_d, k_ds, v_ds, m_ds, o_d, nkbs)
    nc.compile()
    _NC_CACHE[nkbs] = nc
    return nc


def kernel(q, k, v, v_mask, _trace=False, _tmpdir=None):
    q = np.ascontiguousarray(q, dtype=np.float32)
    k = np.ascontiguousarray(k, dtype=np.float32)
    v = np.ascontiguousarray(v, dtype=np.float32)
    v_mask = np.ascontiguousarray(v_mask, dtype=np.float32)
    assert q.shape == (B, LQ, D), q.shape

    # fold the 0/1 mask into k and v on the host (exact; removes the device
    # mask-multiply chain from the critical path)
    k = k * v_mask[:, :, None]
    v = v * v_mask[:, :, None]
    counts = (v_mask > 0.5).sum(axis=1).astype(np.int64)

    if COMPACT:
        # kept key rows first (stable), zero-mask padding after; the packed
        # mask makes padded rows contribute exactly 0 on device.
        order = np.argsort(v_mask <= 0.5, axis=1, kind="stable")
        kc = np.take_along_axis(k, order[:, :, None], axis=1)
        vc = np.take_along_axis(v, order[:, :, None], axis=1)
        mc = np.take_along_axis(v_mask, order, axis=1)
        # sort batches by kept count, largest first; slot 0 takes the top 8
        perm = np.argsort(-counts, kind="stable")
        slot_b = [perm[:NCORES], perm[NCORES:]]
        nkbs = tuple(
            max(1, int(-(-counts[sb].max() // P))) for sb in slot_b
        )
    else:
        kc, vc, mc = k, v, v_mask
        perm = np.arange(B)
        slot_b = [perm[:NCORES], perm[NCORES:]]
        nkbs = (LK // P, LK // P)

    nc = _build_nc(nkbs)
    in_maps = []
    for i in range(NCORES):
        m = {}
        bsel = [slot_b[s][i] for s in range(PB)]
        m["q"] = np.ascontiguousarray(q[bsel])
        for s in range(PB):
            lk = nkbs[s] * P
            bi = slot_b[s][i]
            m[f"k{s}"] = np.ascontiguousarray(kc[bi, :lk])
            m[f"v{s}"] = np.ascontiguousarray(vc[bi, :lk])
            m[f"m{s}"] = np.ascontiguousarray(mc[bi, :lk])
        in_maps.append(m)
    res = bass_utils.run_bass_kernel_spmd(
        nc, in_maps, core_ids=list(range(NCORES)), trace=_trace, tmpdir=_tmpdir
    )
    out = np.empty((B, LQ, D), dtype=np.float32)
    for i in range(NCORES):
        for s in range(PB):
            out[slot_b[s][i]] = res.results[i]["out"][s]
    if _trace:
        kernel.last_results = res
    return out


# revision 6
# speedup vs baseline: 1.3323x; 1.0786x over previous
"""Masked dot-product attention (ESIM masked_softmax) Trainium2 Bass kernel.

Math (per batch):
    s   = q @ k^T ; t = s * m  (== q @ (k*m)^T, exact since m is 0/1)
    p   = exp(t) * m / sum_k(exp(t) * m)   (max-subtraction cancels; |s|<~50
                                            so exp() stays in fp32 range)
    out = p @ v = (exp(t) @ [v*m | m]) -> numerator | denominator

Device mapping (per core, 2 batch slots, data-parallel over 8 cores):
  - masked key rows are compacted away on the host (kept rows first, zero-mask
    padding); batches are sorted by kept-count so slot 0 takes the 8 largest
    (14 key blocks) and slot 1 the 8 smallest (13 blocks for the seed-0 data).
  - scores are computed TRANSPOSED (k on partitions, q free) so exp(s^T) is
    directly the lhsT of the PV matmul; no O(Lq*Lk) transposes.
  - k*m / q are PE-transposed once per slot and copied out of PSUM as fp16;
    the K=64 score matmuls run a SINGLE fp16 pass, row-tiled two-at-a-time.
    (fp16 q/k quantization adds ~5e-3 abs score error -> ~0.5% softmax error.)
  - exp() is split across the Scalar engine (native EXP activation) and the
    Vector engine (2-instruction custom-DVE Schraudolph exp, see dve_exp
    section) so neither engine is the sole bottleneck.
  - PV uses bf16 stationary [v*m | m]: column 64 of the accumulated output is
    the softmax denominator for free.
  - out^T [65, Lq] is PE-transposed back in 128-column chunks and normalized
    with a per-partition reciprocal multiply.
"""

import os
import sys

import numpy as np

sys.path.insert(0, "/opt/trn_rl_repo")

import concourse.bacc as bacc
import concourse.bass as bass  # noqa: F401
import concourse.mybir as mybir
import concourse.tile as tile
from concourse import bass_utils
from concourse import dve_ops
from concourse.dve_ops import DveOp
from concourse.dve_spec import AluOp, Bin, C0, C1, C2, Spec, Src0, Src1, maxx
from concourse.dve_spec import _has_src1
from concourse.dve_spec import lower as dve_lower
from concourse.dve_uop import DveOpSpec
from concourse.masks import make_identity

B, LQ, LK, D = 16, 2048, 2048, 64
NCORES = 8
PB = B // NCORES  # batch slots per core
P = 128
NQB = LQ // P  # 16 q-blocks

COMPACT = os.environ.get("ATT_COMPACT", "1") == "1"
# js whose B-side exp runs on the Vector engine (per half)
DVE_JS = tuple(
    int(x) for x in os.environ.get("ATT_DVE_JS", "").split(",") if x != ""
)

F32 = mybir.dt.float32
F16 = mybir.dt.float16
BF16 = mybir.dt.bfloat16
I32 = mybir.dt.int32
EXP = mybir.ActivationFunctionType.Exp

# --------------------------------------------------------------------------
# Custom DVE exp (2 instructions, ~1.7e-3 max rel err).
# inst1 ANT_EXP_CRUDE: i32out = int(max(x*2^23*log2e + 127*2^23, 2^23))
#   -> Schraudolph bit pattern (127+n)<<23 | f<<23 via int32 convert-on-write.
# inst2 ANT_EXP_FIX reads it bitcast as fp32: crude = 2^n*(1+f):
#   p2n = AND(crude, s0-broadcast 0x7F800000) = 2^n exactly
#   nt  = NOT(p2n) = -(2-eps)*2^(1-n);  gp = crude*nt = -4*(1+f)*(1-2^-24)
#   out = p2n * (Src1 + gp*(imm2 + gp*s1))  [deg-2 minimax of 2^(-gp/4-1)]
# --------------------------------------------------------------------------
LOG2E_SCALED = float(2.0**23 / np.log(2.0))
EXP_BIAS = float(127.0 * 2.0**23)
EXP_FLOOR = float(1.0 * 2.0**23)
EXPMASK_BITS = 0x7F800000
_R0, _R1, _R2 = 0.68127841, -0.01673623, 0.33718486
P_R0 = float(_R0)  # Src1 fill (const term)
P_G2 = float(_R2 / 16.0)  # s1
P_G1 = float(-_R1 / 4.0)  # imm2


def _self_pinned(name, spec, row):
    if name in dve_ops._SUB_OPCODE_FOR_NAME:
        for op in dve_ops.OPS:
            if op.name == name:
                return op
    dve_ops._SUB_OPCODE_FOR_NAME[name] = row
    sha = {}
    for ver in ("v3", "v4"):
        uops = dve_lower(spec, ver=ver)
        sha[ver] = DveOpSpec(
            name=name, opcode=row, uops=uops, rd1_en=_has_src1(spec)
        ).sha(ver)
    op = DveOp(name, spec, subdim=False, uops_sha=sha)
    dve_ops.OPS.append(op)
    dve_ops.CUSTOM_DVE_SPECS[name] = spec
    return op


def _ref_crude(in0, in1, s0, s1, imm2):
    return np.maximum(
        in0.astype(np.float32) * np.float32(s0) + np.float32(s1), np.float32(imm2)
    )


def _ref_fix(in0, in1, s0, s1, imm2):
    ci = np.ascontiguousarray(in0, dtype=np.float32).view(np.int32)
    mask_i = np.ascontiguousarray(np.asarray(s0, np.float32).reshape(-1, 1)).view(
        np.int32
    )
    p2n_bits = ci & mask_i
    p2n = p2n_bits.view(np.float32)
    nt = (~p2n_bits).view(np.float32)
    gp = in0.astype(np.float32) * nt
    r0 = in1.astype(np.float32).reshape(in0.shape)
    return p2n * (r0 + gp * (np.float32(imm2) + gp * np.float32(s1)))


EXP_CRUDE = _self_pinned(
    "ANT_EXP_CRUDE", Spec(body=maxx(Src0 * C0 + C1, C2), reference=_ref_crude), row=17
)
_p2n = Bin(AluOp.BITWISE_AND, Src0, C0)
_nt = Bin(AluOp.BITWISE_NOT, _p2n, _p2n)
_gp = Src0 * _nt
EXP_FIX = _self_pinned(
    "ANT_EXP_FIX",
    Spec(body=_p2n * (Src1 + _gp * (C2 + _gp * C1)), reference=_ref_fix),
    row=18,
)


class _BatchCtx:
    pass


def _attention_core(tc, q_d, k_ds, v_ds, m_ds, o_d, nkbs):
    """Emit the per-core program. k/v/m dram handles and nkb per batch slot."""
    nc = tc.nc
    pools = []

    def pool(name, bufs, space="SBUF"):
        p = tc.alloc_tile_pool(name=name, bufs=bufs, space=space)
        pools.append(p)
        return p

    singles = pool("singles", 1)
    stage = pool("stage", 2)
    main = pool("main", 2)
    wtp = pool("wt", 12)
    crp = pool("crp", 2)
    outp = pool("outp", 2)
    smalls = pool("smalls", 4)

    ps_s = pool("ps_s", 3, space="PSUM")  # 3 x [128,1024] = 6 banks
    ps_pv = pool("ps_pv", 2, space="PSUM")  # 2 x 1-bank slots (pv chunks + nat)

    ident = singles.tile([P, P], F32, tag="ident")
    if DVE_JS:
        emask = singles.tile([P, 1], I32, tag="emask")
        r0t = singles.tile([P, 1024], F32, tag="r0t")

    def emit_exp_dve(w_ap, s_ap, tag):
        crude = crp.tile([P, 1024], I32, tag="crude", name=f"cr{tag}")
        nc.vector._custom_dve(
            EXP_CRUDE, out=crude, in0=s_ap,
            s0=LOG2E_SCALED, s1=EXP_BIAS, imm2=EXP_FLOOR,
        )
        nc.vector._custom_dve(
            EXP_FIX, out=w_ap, in0=crude.bitcast(F32),
            in1=r0t, s0=emask.bitcast(F32), s1=P_G2, imm2=P_G1,
        )

    def prep_io(b, use_act_ring=False):
        nkb = nkbs[b]
        npair = (nkb + 1) // 2
        bc = _BatchCtx()
        bc.nkb, bc.npair = nkb, npair
        # partition-major layouts: key/query index = p*nblocks + t, so every
        # DMA is one contiguous run per partition (128 descriptors, full BW).
        ksrc = k_ds[b].rearrange("(p t) d -> p t d", p=P)
        qsrc = q_d[b].rearrange("(p t) d -> p t d", p=P)
        bc.knat = stage.tile([P, nkb, D], F32, tag="knat", name=f"knat{b}")
        bc.qnat = stage.tile([P, NQB, D], F32, tag="qnat", name=f"qnat{b}")
        bc.vnat = stage.tile([P, nkb, D], F32, tag="vnat", name=f"vnat{b}")
        bc.m_sb = stage.tile([P, nkb], F32, tag="m", name=f"m_sb{b}")
        h0 = min(2 * ((npair + 1) // 2), nkb)
        if use_act_ring:
            # slot-0 head fast path: tiny dedicated loads for the first
            # k-pair and first 4 q-blocks so the first S matmul issues early.
            bc.kf = stage.tile([P, 2, D], F32, tag="kf", name=f"kf{b}")
            nc.sync.dma_start(out=bc.kf, in_=ksrc[:, 0:2, :])
            nc.sync.dma_start(out=bc.qnat[:, 0:4, :], in_=qsrc[:, 0:4, :])
            nc.sync.dma_start(out=bc.qnat[:, 4:8, :], in_=qsrc[:, 4:8, :])
            # second q half on the idle ACT HWDGE ring, parallel. Later slots
            # must not touch the ACT ring (it would stall exp).
            nc.scalar.dma_start(out=bc.qnat[:, 8:, :], in_=qsrc[:, 8:, :])
        else:
            nc.sync.dma_start(out=bc.qnat[:, 0:8, :], in_=qsrc[:, 0:8, :])
            nc.sync.dma_start(out=bc.qnat[:, 8:, :], in_=qsrc[:, 8:, :])
        nc.gpsimd.dma_start(out=bc.knat[:, :h0, :], in_=ksrc[:, :h0, :])
        nc.gpsimd.dma_start(out=bc.knat[:, h0:, :], in_=ksrc[:, h0:, :])
        nc.gpsimd.dma_start(
            out=bc.vnat, in_=v_ds[b].rearrange("(p t) d -> p t d", p=P)
        )
        nc.gpsimd.dma_start(out=bc.m_sb, in_=m_ds[b].rearrange("(p t) -> p t", p=P))
        return bc

    def prep_units(b, bc):
        """Closures emitting prep compute; callable in order, spreadable."""
        nkb, npair = bc.nkb, bc.npair
        km = bc.knat  # host pre-multiplied: knat already is k*m
        bc.kmT = main.tile([P, npair, P], F16, tag="kmT", name=f"kmT{b}")
        bc.qT = main.tile([P, LQ], F16, tag="qT", name=f"qT{b}")
        bc.vme = stage.tile([P, nkb, D + 1], BF16, tag="vme", name=f"vme{b}")
        bc.out_sb = outp.tile([P, NQB, D], F32, tag="osb", name=f"osb{b}")

        h0 = 2 * ((npair + 1) // 2)
        fast = hasattr(bc, "kf")

        def u_fast():
            # first k-pair + first 4 q-blocks: unblocks S(j=0, c=0) early
            tr = ps_s.tile([P, 5 * P], F32, tag="s", name=f"trf{b}")
            nc.tensor.transpose(tr[:, 0:P], bc.kf, ident)
            for i in range(4):
                cs = slice((i + 1) * P, (i + 2) * P)
                nc.tensor.transpose(tr[0:64, cs], bc.qnat[:, i, :], ident)
            dst = bc.kmT[:, 0:1, :].rearrange("p a b -> p (a b)")
            nc.scalar.copy(dst, tr[:, 0:P])
            nc.scalar.copy(bc.qT[0:64, 0:512], tr[0:64, P:])
            # row-pairing needs q^T duplicated into partitions 64-127:
            # cheap on-chip DMA (64 descriptors), off the critical path.
            nc.sync.dma_start(out=bc.qT[64:128, 0:512], in_=bc.qT[0:64, 0:512])

        def u_kmT(grp, act_copy=False):
            jlo = grp * (npair + 1) // 2
            jhi = npair if grp else (npair + 1) // 2
            if fast and grp == 0:
                jlo = 1

            def go():
                nj = jhi - jlo
                if nj <= 0:
                    return
                tr = ps_s.tile([P, nj * P], F32, tag="s", name=f"trk{b}_{grp}")
                for j in range(jlo, jhi):
                    blk = slice(2 * j, min(2 * j + 2, nkb))
                    rows = slice(0, 64 * (blk.stop - blk.start))
                    nc.tensor.transpose(
                        tr[rows, (j - jlo) * P : (j - jlo + 1) * P],
                        km[:, blk, :], ident,
                    )
                dst = bc.kmT[:, jlo:jhi, :].rearrange("p a b -> p (a b)")
                (nc.scalar.copy if act_copy else nc.vector.tensor_copy)(dst, tr)

            return go

        def u_qT(g, ilo, ihi, act_copy=False):
            def go():
                tr = ps_s.tile(
                    [P, (ihi - ilo) * P], F32, tag="s", name=f"trq{b}_{g}_{ilo}"
                )
                for i in range(ilo, ihi):
                    t = g * 8 + i
                    cs = slice((i - ilo) * P, (i - ilo + 1) * P)
                    nc.tensor.transpose(tr[0:64, cs], bc.qnat[:, t, :], ident)
                half = slice((g * 8 + ilo) * P, (g * 8 + ihi) * P)
                (nc.scalar.copy if act_copy else nc.vector.tensor_copy)(
                    bc.qT[0:64, half], tr[0:64, :]
                )
                nc.sync.dma_start(out=bc.qT[64:128, half], in_=bc.qT[0:64, half])

            return go

        def u_vme():
            nc.gpsimd.tensor_copy(bc.vme[:, :, 0:D], bc.vnat)
            nc.gpsimd.tensor_copy(bc.vme[:, :, D], bc.m_sb[:, :])

        units = [
            u_kmT(0, act_copy=fast), u_qT(0, 4 if fast else 0, 8, act_copy=fast),
            u_kmT(1), u_vme, u_qT(1, 0, 4), u_qT(1, 4, 8),
        ]
        if fast:
            units.insert(0, u_fast)
        else:
            units.insert(1, u_qT(0, 0, 4))
            units[2] = u_qT(0, 4, 8)
        return units

    def main_half(b, bc, h, side_work=(), finals_out=None):
        nkb, npair = bc.nkb, bc.npair
        odd = nkb % 2  # last pair has only an A block
        side = list(side_work)
        pvc = [
            ps_pv.tile([D + 1, 512], F32, tag="pv", name=f"pv{b}_{h}_{c}")
            for c in range(2)
        ]

        def emit_pv(j, wA, wB):
            # c innermost: consecutive matmuls alternate PSUM banks, so the
            # accumulate never waits on its own bank's drain.
            kbs = [(2 * j, wA)]
            if wB is not None:
                kbs.append((2 * j + 1, wB))
            for kb, w in kbs:
                for c in range(2):
                    cs = slice(c * 512, (c + 1) * 512)
                    nc.tensor.matmul(
                        pvc[c], bc.vme[:, kb, :], w[:, cs],
                        start=(kb == 0), stop=(kb == nkb - 1),
                    )

        pend = []
        for j in range(npair):
            jodd = odd and j == npair - 1
            sA = ps_s.tile([P, 1024], F32, tag="s", name=f"sA{b}_{h}_{j}")
            sB = None if jodd else ps_s.tile([P, 1024], F32, tag="s", name=f"sB{b}_{h}_{j}")
            # c innermost: consecutive same-side matmuls alternate banks (no
            # accumulate drain-wait) and A/B stay adjacent so they row-pair.
            for c in range(2):
                qs = slice(h * 1024 + c * 512, h * 1024 + (c + 1) * 512)
                cs = slice(c * 512, (c + 1) * 512)
                nc.tensor.matmul(
                    sA[:, cs], bc.kmT[0:64, j, :], bc.qT[0:64, qs],
                    start=True, stop=True, tile_position=(0, 0),
                )
                if not jodd:
                    nc.tensor.matmul(
                        sB[:, cs], bc.kmT[64:128, j, :], bc.qT[64:128, qs],
                        start=True, stop=True, tile_position=(64, 0),
                    )
            wA = wtp.tile([P, 1024], BF16, tag="wt", name=f"wA{b}_{h}_{j}")
            wB = None if jodd else wtp.tile([P, 1024], BF16, tag="wt", name=f"wB{b}_{h}_{j}")
            # B-side exp of designated js runs on the Vector engine; emitting
            # it first keeps the ACT stream dense. PV lags two j-groups so the
            # in-order PE never stalls on exp latency here.
            if (not jodd) and j in DVE_JS:
                emit_exp_dve(wB, sB, f"{b}_{h}_{j}")
                nc.scalar.activation(out=wA, in_=sA, func=EXP)
            else:
                nc.scalar.activation(out=wA, in_=sA, func=EXP)
                if not jodd:
                    nc.scalar.activation(out=wB, in_=sB, func=EXP)
            pend.append((j, wA, wB))
            if len(pend) > 2:
                emit_pv(*pend.pop(0))
            if side:
                side.pop(0)()
        while pend:
            emit_pv(*pend.pop(0))
        while side:
            side.pop(0)()

        # drain: copy the accumulators out (freeing the pv slots for the next
        # half) and hand the transpose-back/normalize work to the caller so it
        # can interleave into the next half's stream instead of starving ACT.
        outT = outp.tile([D + 1, 1024], F32, tag="outT", name=f"outT{b}_{h}")
        for c in range(2):
            nc.vector.tensor_copy(outT[:, c * 512 : (c + 1) * 512], pvc[c])

        def fin(q0):
            def go():
                nat4 = ps_s.tile([P, 4, D + 1], F32, tag="s", name=f"nat{b}_{h}_{q0}")
                for i in range(4):
                    nc.tensor.transpose(
                        nat4[:, i, :], outT[:, (q0 + i) * P : (q0 + i + 1) * P],
                        ident[0 : D + 1, 0 : D + 1],
                    )
                rc4 = smalls.tile([P, 4], F32, tag="rc", name=f"rc{b}_{h}_{q0}")
                nc.vector.reciprocal(rc4, nat4[:, :, D])
                for i in range(4):
                    nc.vector.tensor_scalar_mul(
                        bc.out_sb[:, h * 8 + q0 + i, :], nat4[:, i, 0:D],
                        rc4[:, i : i + 1],
                    )

            return go

        if finals_out is None:
            fin(0)()
            fin(4)()
        else:
            finals_out.extend([fin(0), fin(4)])

    def store(b, bc, h=None):
        dst = o_d[b].rearrange("(p t) d -> p t d", p=P)
        if h is None:
            nc.sync.dma_start(out=dst, in_=bc.out_sb)
        else:
            hs = slice(h * 8, (h + 1) * 8)
            nc.sync.dma_start(out=dst[:, hs, :], in_=bc.out_sb[:, hs, :])

    # Interleave slot 1's prep into slot 0's stream: no PE bubble at the
    # boundary, and prep transposes spread out so HAM stays warm. Only the
    # units needed by the first few S matmuls run before the main loop; the
    # rest spread as per-iteration side work.
    bcs = [prep_io(0, use_act_ring=True)]
    # DMA issue first; engine-local init (identity, exp consts) after, so the
    # DGE rings start streaming input bytes immediately.
    make_identity(nc, ident)
    if DVE_JS:
        nc.gpsimd.memset(emask, EXPMASK_BITS)
        nc.gpsimd.memset(r0t, P_R0)
    # u0: [fast, kmT0, qT0b, kmT1, vme, qT1a, qT1b]
    u0 = prep_units(0, bcs[0])
    for u in u0[:3]:
        u()
    u0[4]()  # vme0 early (gpsimd; PV(j=0) needs it)
    if PB > 1:
        bcs.append(prep_io(1))
        # u1: [kmT0, qT0a, qT0b, kmT1, vme, qT1a, qT1b]
        u1 = prep_units(1, bcs[1])
    else:
        u1 = []
    side00 = [u0[3], u0[5], u0[6]] + u1[:1]
    f = []
    main_half(0, bcs[0], 0, side_work=side00, finals_out=f)
    f2 = []
    main_half(0, bcs[0], 1, side_work=f + u1[1:4], finals_out=f2)
    if PB > 1:
        u1[4]()  # vme1 (gpsimd; PV(b1, j=0) needs it)
        f3 = []
        main_half(1, bcs[1], 0, side_work=f2 + u1[5:], finals_out=f3)
        store(0, bcs[0])
        main_half(
            1, bcs[1], 1,
            side_work=list(f3) + [lambda: store(1, bcs[1], 0)],
            finals_out=None,
        )
        store(1, bcs[1], 1)
    else:
        for u in f2:
            u()
        store(0, bcs[0])

    for p in reversed(pools):
        p.release()


_NC_CACHE = {}


def _build_nc(nkbs):
    nkbs = tuple(nkbs)
    if nkbs in _NC_CACHE:
        return _NC_CACHE[nkbs]
    nc = bacc.Bacc(None, target_bir_lowering=False, debug=False)
    q_d = nc.dram_tensor("q", [PB, LQ, D], F32, kind="ExternalInput")
    k_ds, v_ds, m_ds = [], [], []
    for s, nkb in enumerate(nkbs):
        lk = nkb * P
        k_ds.append(nc.dram_tensor(f"k{s}", [lk, D], F32, kind="ExternalInput"))
        v_ds.append(nc.dram_tensor(f"v{s}", [lk, D], F32, kind="ExternalInput"))
        m_ds.append(nc.dram_tensor(f"m{s}", [lk], F32, kind="ExternalInput"))
    o_d = nc.dram_tensor("out", [PB, LQ, D], F32, kind="ExternalOutput")
    with tile.TileContext(nc) as tc:
        _attention_core(tc, q_d, k_ds, v_ds, m_ds, o_d, nkbs)
    nc.compile()
    _NC_CACHE[nkbs] = nc
    return nc


def kernel(q, k, v, v_mask, _trace=False, _tmpdir=None):
    q = np.ascontiguousarray(q, dtype=np.float32)
    k = np.ascontiguousarray(k, dtype=np.float32)
    v = np.ascontiguousarray(v, dtype=np.float32)
    v_mask = np.ascontiguousarray(v_mask, dtype=np.float32)
    assert q.shape == (B, LQ, D), q.shape

    # fold the 0/1 mask into k and v on the host (exact; removes the device
    # mask-multiply chain from the critical path)
    k = k * v_mask[:, :, None]
    v = v * v_mask[:, :, None]
    counts = (v_mask > 0.5).sum(axis=1).astype(np.int64)

    if COMPACT:
        # kept key rows first (stable), zero-mask padding after; the packed
        # mask makes padded rows contribute exactly 0 on device.
        order = np.argsort(v_mask <= 0.5, axis=1, kind="stable")
        kc = np.take_along_axis(k, order[:, :, None], axis=1)
        vc = np.take_along_axis(v, order[:, :, None], axis=1)
        mc = np.take_along_axis(v_mask, order, axis=1)
        # sort batches by kept count, largest first; slot 0 takes the top 8
        perm = np.argsort(-counts, kind="stable")
        slot_b = [perm[:NCORES], perm[NCORES:]]
        nkbs = tuple(
            max(1, int(-(-counts[sb].max() // P))) for sb in slot_b
        )
    else:
        kc, vc, mc = k, v, v_mask
        perm = np.arange(B)
        slot_b = [perm[:NCORES], perm[NCORES:]]
        nkbs = (LK // P, LK // P)

    nc = _build_nc(nkbs)
    in_maps = []
    for i in range(NCORES):
        m = {}
        bsel = [slot_b[s][i] for s in range(PB)]
        m["q"] = np.ascontiguousarray(q[bsel])
        for s in range(PB):
            lk = nkbs[s] * P
            bi = slot_b[s][i]
            m[f"k{s}"] = np.ascontiguousarray(kc[bi, :lk])
            m[f"v{s}"] = np.ascontiguousarray(vc[bi, :lk])
            m[f"m{s}"] = np.ascontiguousarray(mc[bi, :lk])
        in_maps.append(m)
    res = bass_utils.run_bass_kernel_spmd(
        nc, in_maps, core_ids=list(range(NCORES)), trace=_trace, tmpdir=_tmpdir
    )
    out = np.empty((B, LQ, D), dtype=np.float32)
    for i in range(NCORES):
        for s in range(PB):
            out[slot_b[s][i]] = res.results[i]["out"][s]
    if _trace:
        kernel.last_results = res
    return out
